# revision 1
# baseline (speedup 1.0000x reference)
"""DiffeomorphismNet fused kernel for 8x TRN2 NeuronCores (data parallel).

Math (per sample row x = [xt | xtdot | xz], each 64 wide):
  branch(v):  h0 = W_in v + b_in;  h_{i+1} = relu(W_h[i] h_i + b_h[i]), i=0..2
              D_{i+1} = (W_h[i] h_{i+1} + b_h[i] > 0)        # the module's quirk
  out cols  0:64   h_out  = W_out h3_t + b_out                       (t branch)
  out cols 64:128  h_dot  = W_out D3t W2 D2t W1 D1t (W_h0 W_in) xtdot (t branch,
                   computed as a vector chain -- no Jacobian materialization)
  out cols 128:192 zng    = row_norms(W_out D3z W2 D2z W1 D1z G0)     (z branch,
                   G0 = W_h0 @ W_in; per-sample [512,64] Jacobian chain)

Sharding: batch 4096 -> 8 cores x 512. Weights replicated.
Layouts on device (per core, B=512 local samples):
  activations/masks: [128 part, 4 chunk, 512 samples]  (hidden dim = chunk*128+p)
  Jacobian J: groups of 8 samples -> [128 part, 4 chunk, 8*64 cols] fp32r

Precision strategy:
  - fp32r (fp32 with an 11-bit mantissa) streams at 1 cycle/row on the PE;
    plain fp32 takes 4.  Mask z-values must be near-fp32-exact (they are
    thresholded at 0), so the forward/delta matmuls run as THREE fp32r passes
    with hi/lo-split operands:  W@h ~ Whi@hhi + Wlo@hhi + Whi@hlo  (the
    dropped Wlo@hlo term is ~2^-24 relative) -- 3 cycles/row, error ~2e-7.
  - The Jacobian chain and h_dot chain are continuous outputs; single-pass
    fp32r (~1e-4 relative) is plenty.  Toggle with JAC_F32R=0 -> all-fp32.
"""

import os
import sys

sys.path.insert(0, "/opt/trn_rl_repo")

import numpy as np
import concourse.bass as bass
import concourse.tile as tile
from concourse import bacc
from concourse import mybir
from concourse.bass_utils import run_bass_kernel_spmd

N_CORES = 8
B = 4096
BC = B // N_CORES          # 512 samples per core
D = 64                     # n
H = 512                    # hidden
NL = 3                     # hidden layers
NMC = H // 128             # partition chunks of the hidden dim
NG = BC // 8               # jacobian groups of 8 samples
JN = 8 * D                 # 512 columns per jacobian group

F32 = mybir.dt.float32
F32R = mybir.dt.float32r

JAC_F32R = os.environ.get("JAC_F32R", "1") == "1"

ADD = mybir.AluOpType.add
MAX = mybir.AluOpType.max
MULT = mybir.AluOpType.mult
SUB = mybir.AluOpType.subtract
ISGT = mybir.AluOpType.is_gt

# cb (fp32) column offsets
_O_WOUTR = 0                 # W_out^T as [128, 4, 64]
_O_BIN = 256                 # [128, 4]
_O_BH = 260                  # [128, 12]
_O_BOUTR = 272               # [128, 64]
_O_ID = 336                  # [128, 128] identity
CB_COLS = 464
# cbr1a (fp32r): layer-0 hi weights + W_in (for G0) + W_in^T hi/lo
_O_RWT0 = 0                  # hi(W_h[0]^T)  [128, 4, 512]
_O_WINR = 2048               # hi(W_in) as [128, 4, 64]
_O_WINTH = 2304              # hi(W_in^T) rows 0:64, [64, 512]
_O_WINTL = 2816              # lo(W_in^T) rows 0:64, [64, 512]
CBR1A_COLS = 3328
# cbr1b (fp32r): layer-0 lo weights
_O_WLO0 = 0
CBR1B_COLS = 2048
# cbr2a (fp32r): layer 1
_O_RWT1 = 0
_O_WLO1 = 2048
CBR2A_COLS = 4096
# cbr2b (fp32r): layer 2 + W_out^T
_O_RWT2 = 0
_O_WLO2 = 2048
_O_RWOUT = 4096              # hi(W_out^T) as [128, 4, 64]
CBR2B_COLS = 4352


def _round_fp32r(x: np.ndarray) -> np.ndarray:
    """Round-to-nearest-even to 11 explicit mantissa bits (fp32r grid)."""
    u = x.astype(np.float32).view(np.uint32).astype(np.uint64)
    keep = np.uint64(0xFFFFF000)
    half = np.uint64(0x800)
    lsb = (u >> np.uint64(12)) & np.uint64(1)
    r = (u + half - np.uint64(1) + lsb) & keep
    return r.astype(np.uint32).view(np.float32)


def _build():
    nc = bacc.Bacc("TRN2", target_bir_lowering=False, debug=False,
                   num_devices=N_CORES)

    cb_d = nc.dram_tensor("cb", [128, CB_COLS], F32, kind="ExternalInput")
    rdt = F32R if JAC_F32R else F32
    cbr1a_d = nc.dram_tensor("cbr1a", [128, CBR1A_COLS], rdt, kind="ExternalInput")
    cbr1b_d = nc.dram_tensor("cbr1b", [128, CBR1B_COLS], rdt, kind="ExternalInput")
    cbr2a_d = nc.dram_tensor("cbr2a", [128, CBR2A_COLS], rdt, kind="ExternalInput")
    cbr2b_d = nc.dram_tensor("cbr2b", [128, CBR2B_COLS], rdt, kind="ExternalInput")
    xbh_d = nc.dram_tensor("xbh", [64, 3 * BC], rdt, kind="ExternalInput")
    xbl_d = nc.dram_tensor("xbl", [64, 3 * BC], rdt, kind="ExternalInput")
    out_d = nc.dram_tensor("out", [BC, 3 * D], F32, kind="ExternalOutput")

    with tile.TileContext(nc) as tc:
        with (
            tc.tile_pool(name="const", bufs=1) as pc,
            tc.tile_pool(name="act", bufs=2) as pa,
            tc.tile_pool(name="mask", bufs=1) as pm,
            tc.tile_pool(name="jac", bufs=2) as pj,
            tc.tile_pool(name="ps", bufs=4, space="PSUM") as ps,
            tc.tile_pool(name="psa", bufs=1, space="PSUM") as psa,
            tc.tile_pool(name="pso", bufs=2, space="PSUM") as pso,
        ):
            cb = pc.tile([128, CB_COLS], F32)
            cbr1a = pc.tile([128, CBR1A_COLS], rdt)
            cbr1b = pc.tile([128, CBR1B_COLS], rdt)
            cbr2a = pc.tile([128, CBR2A_COLS], rdt)
            cbr2b = pc.tile([128, CBR2B_COLS], rdt)
            xbh = pc.tile([64, 3 * BC], rdt)
            xbl = pc.tile([64, 3 * BC], rdt)
            nc.sync.dma_start(cbr1a[:], cbr1a_d.ap())
            nc.sync.dma_start(cb[:], cb_d.ap())
            nc.sync.dma_start(xbh[:], xbh_d.ap())
            nc.sync.dma_start(xbl[:], xbl_d.ap())
            nc.sync.dma_start(cbr1b[:], cbr1b_d.ap())
            nc.sync.dma_start(cbr2a[:], cbr2a_d.ap())
            nc.sync.dma_start(cbr2b[:], cbr2b_d.ap())

            # DVE warm-up: observe each input DMA semaphore once (keeps the
            # auto-generated wait-split event semaphores to a minimum).
            warm = pc.tile([128, 4], F32)
            nc.vector.tensor_copy(warm[0:1, 0:1], cb[0:1, 0:1])
            nc.vector.tensor_copy(warm[0:1, 1:2].bitcast(rdt), cbr1a[0:1, 0:1])
            nc.vector.tensor_copy(warm[0:1, 1:2].bitcast(rdt), cbr1b[0:1, 0:1])
            nc.vector.tensor_copy(warm[0:1, 2:3].bitcast(rdt), cbr2a[0:1, 0:1])
            nc.vector.tensor_copy(warm[0:1, 3:4].bitcast(rdt), cbr2b[0:1, 0:1])
            nc.vector.tensor_copy(warm[0:1, 0:1].bitcast(rdt), xbh[0:1, 0:1])
            nc.vector.tensor_copy(warm[0:1, 1:2].bitcast(rdt), xbl[0:1, 0:1])

            WoutR = cb[:, _O_WOUTR:_O_WOUTR + 256].rearrange(
                "p (kc n) -> p kc n", kc=4)
            bin_ = cb[:, _O_BIN:_O_BIN + 4]
            bh = cb[:, _O_BH:_O_BH + 12]
            boutR = cb[:, _O_BOUTR:_O_BOUTR + 64]
            ident = cb[:, _O_ID:_O_ID + 128]

            def r3(ap, off):
                return ap[:, off:off + 2048].rearrange("p (kc m) -> p kc m", kc=4)

            WHI = [r3(cbr1a, _O_RWT0), r3(cbr2a, _O_RWT1), r3(cbr2b, _O_RWT2)]
            WLO = [r3(cbr1b, _O_WLO0), r3(cbr2a, _O_WLO1), r3(cbr2b, _O_WLO2)]
            WinR = cbr1a[:, _O_WINR:_O_WINR + 256].rearrange(
                "p (kc n) -> p kc n", kc=4)
            RWout = cbr2b[:, _O_RWOUT:_O_RWOUT + 256].rearrange(
                "p (kc n) -> p kc n", kc=4)
            WinTH = cbr1a[0:64, _O_WINTH:_O_WINTH + 512]
            WinTL = cbr1a[0:64, _O_WINTL:_O_WINTL + 512]

            xtTh, xtTl = xbh[:, 0:BC], xbl[:, 0:BC]
            xdTh = xbh[:, BC:2 * BC]
            xzTh, xzTl = xbh[:, 2 * BC:3 * BC], xbl[:, 2 * BC:3 * BC]

            # PE warm-up: observe the cbr1 DMA semaphore from a fresh PSUM slot
            pwarm = ps.tile([128, D], F32, tag="ps")
            nc.tensor.matmul(pwarm[:], WHI[0][:, 0, 0:128], WHI[0][:, 0, 0:D],
                             start=True, stop=True)

            # ---- G0 = W_h[0] @ W_in  [512, 64] as [128, 4(mc), 64] ----
            # single-pass fp32r: feeds only the zng Jacobian chain
            G0 = pc.tile([128, 4, D], F32)
            for mc in range(NMC):
                pg = pso.tile([128, D], F32, tag="po")
                for kc in range(NMC):
                    nc.tensor.matmul(pg[:], WHI[0][:, kc, mc * 128:(mc + 1) * 128],
                                     WinR[:, kc, :], start=(kc == 0),
                                     stop=(kc == NMC - 1))
                nc.vector.tensor_copy(G0[:, mc, :], pg[:])

            def mm3(psum, i, mc, hhi, hlo):
                """z[mc] += W_h[i] @ h via 3 fp32r passes (12 matmuls);
                hi*hi passes first so the lo-weight DMA can trail."""
                sl = slice(mc * 128, (mc + 1) * 128)
                n = 0
                for wop, rop in ((WHI[i], hhi), (WLO[i], hhi), (WHI[i], hlo)):
                    for kc in range(NMC):
                        nc.tensor.matmul(psum[:], wop[:, kc, sl], rop[:, kc, :],
                                         start=(n == 0), stop=(n == 11))
                        n += 1

            def split(hf, hhi, hlo, mc):
                """hhi = round12(hf);  hlo = round12(hf - hhi)  (DVE)."""
                nc.vector.tensor_copy(hhi[:, mc, :], hf[:, mc, :])
                nc.vector.tensor_tensor(hlo[:, mc, :], hf[:, mc, :],
                                        hhi[:, mc, :].bitcast(F32), SUB)

            def forward(xTh, xTl, tagpfx, keep_h3=False):
                """Forward on x^T hi/lo [64, BC]; returns (h3_f32|None, masks)."""
                hf = pa.tile([128, 4, BC], F32, tag="h", name=f"h0{tagpfx}")
                hhi = pa.tile([128, 4, BC], rdt, tag="hhi", name=f"h0h{tagpfx}")
                hlo = pa.tile([128, 4, BC], rdt, tag="hlo", name=f"h0l{tagpfx}")
                for mc in range(NMC):
                    p0 = ps.tile([128, BC], F32, tag="ps", name=f"p0{tagpfx}")
                    sl = slice(mc * 128, (mc + 1) * 128)
                    for n, (w, r) in enumerate(((WinTH, xTh), (WinTL, xTh),
                                                (WinTH, xTl))):
                        nc.tensor.matmul(p0[:], w[:, sl], r,
                                         start=(n == 0), stop=(n == 2))
                    # h0 = z + b_in  (no relu on the input layer)
                    nc.vector.tensor_scalar_add(hf[:, mc, :], p0[:],
                                                bin_[:, mc:mc + 1])
                    split(hf, hhi, hlo, mc)
                masks = []
                for i in range(NL):
                    last = (i == NL - 1)
                    hfn = pa.tile([128, 4, BC], F32, tag="h", name=f"h{i+1}{tagpfx}")
                    hhin = pa.tile([128, 4, BC], rdt, tag="hhi",
                                   name=f"h{i+1}h{tagpfx}")
                    hlon = pa.tile([128, 4, BC], rdt, tag="hlo",
                                   name=f"h{i+1}l{tagpfx}")
                    for mc in range(NMC):
                        pz = ps.tile([128, BC], F32, tag="ps", name=f"pz{tagpfx}")
                        mm3(pz, i, mc, hhi, hlo)
                        # h_{i+1} = relu(z + b) = max(z + b, 0)
                        nc.vector.tensor_scalar(
                            hfn[:, mc, :], pz[:],
                            bh[:, 4 * i + mc:4 * i + mc + 1], 0.0, ADD, MAX)
                        split(hfn, hhin, hlon, mc)
                    hf, hhi, hlo = hfn, hhin, hlon
                    Dm = pm.tile([128, 4, BC], F32, tag=f"D{i}",
                                 name=f"D{i}{tagpfx}")
                    for mc in range(NMC):
                        pd = ps.tile([128, BC], F32, tag="ps", name=f"pd{tagpfx}")
                        mm3(pd, i, mc, hhi, hlo)
                        # D = (W h_{i+1} + b > 0)
                        nc.vector.tensor_scalar(
                            Dm[:, mc, :], pd[:],
                            bh[:, 4 * i + mc:4 * i + mc + 1], 0.0, ADD, ISGT)
                    masks.append(Dm)
                return (hf if keep_h3 else None), masks

            # ---- output staging tiles, one per 128-sample block ----
            O = [pc.tile([128, 3 * D], F32, tag=f"O{g}", name=f"O{g}")
                 for g in range(4)]

            # ================= t branch =================
            h3t, Dt = forward(xtTh, xtTl, "t", keep_h3=True)

            # h_out (sample-major, exact fp32): out[s,n] = h3t^T W_out^T + b_out
            for mg in range(4):
                po = pso.tile([128, D], F32, tag="po")
                for kc in range(NMC):
                    nc.tensor.matmul(po[:],
                                     h3t[:, kc, mg * 128:(mg + 1) * 128],
                                     WoutR[:, kc, :], start=(kc == 0),
                                     stop=(kc == NMC - 1))
                nc.vector.tensor_add(O[mg][:, 0:D], po[:], boutR)

            # h_dot chain, single-pass fp32r: v = W_h0 (W_in xdT); v = Di*(W v)
            w0r = pa.tile([128, 4, BC], rdt, tag="hhi", name="w0r")
            for mc in range(NMC):
                pw = ps.tile([128, BC], F32, tag="ps", name="pw")
                nc.tensor.matmul(pw[:], WinTH[:, mc * 128:(mc + 1) * 128], xdTh,
                                 start=True, stop=True)
                nc.vector.tensor_copy(w0r[:, mc, :], pw[:])
            v = w0r
            for i in range(NL):
                vn = pa.tile([128, 4, BC], rdt, tag="hlo", name=f"v{i+1}")
                for mc in range(NMC):
                    pv = ps.tile([128, BC], F32, tag="ps", name="pv")
                    for kc in range(NMC):
                        nc.tensor.matmul(pv[:],
                                         WHI[i][:, kc, mc * 128:(mc + 1) * 128],
                                         v[:, kc, :], start=(kc == 0),
                                         stop=(kc == NMC - 1))
                    nc.vector.tensor_mul(vn[:, mc, :], pv[:], Dt[i][:, mc, :])
                v = vn
            for mg in range(4):
                po = pso.tile([128, D], F32, tag="po")
                for kc in range(NMC):
                    nc.tensor.matmul(po[:], v[:, kc, mg * 128:(mg + 1) * 128],
                                     RWout[:, kc, :], start=(kc == 0),
                                     stop=(kc == NMC - 1))
                nc.vector.tensor_copy(O[mg][:, D:2 * D], po[:])

            # ================= z branch =================
            _, Dz = forward(xzTh, xzTl, "z")

            zng2 = pc.tile([64, BC], F32)
            zngT = pc.tile([64, BC], F32)

            def zng_flush(mg):
                c0 = mg * 128
                nc.scalar.sqrt(zngT[:, c0:c0 + 128], zng2[:, c0:c0 + 128])
                pt = pso.tile([128, 64], F32, tag="pt", bufs=1)
                nc.tensor.transpose(pt[:], zngT[:, c0:c0 + 128],
                                    ident[0:64, 0:64])
                nc.vector.tensor_copy(O[mg][:, 2 * D:3 * D], pt[:])

            for g in range(NG):
                s0 = g * 8
                # J0[p, kc, b, d] = G0[p, kc, d] * D1z[p, kc, s0+b]
                # (on GpSimd: pure-SBUF op, keeps DVE free for the step masks)
                J0 = pj.tile([128, 4, 8, D], rdt, tag="J0", name="J0")
                nc.gpsimd.tensor_tensor(
                    J0[:],
                    G0[:, :, None, :].broadcast_to([128, 4, 8, D]),
                    Dz[0][:, :, s0:s0 + 8][:, :, :, None]
                    .broadcast_to([128, 4, 8, D]), MULT)
                J = J0
                for i in (1, 2):
                    Jn = pj.tile([128, 4, 8, D], rdt, tag=f"J{i}", name=f"J{i}",
                                 bufs=1)
                    for mc in range(NMC):
                        pjm = ps.tile([128, JN], F32, tag="ps", name="pjm")
                        for kc in range(NMC):
                            nc.tensor.matmul(
                                pjm[:],
                                WHI[i][:, kc, mc * 128:(mc + 1) * 128],
                                J[:, kc, :, :].rearrange("p b d -> p (b d)"),
                                start=(kc == 0), stop=(kc == NMC - 1))
                        nc.vector.tensor_tensor(
                            Jn[:, mc, :, :],
                            pjm[:].rearrange("p (b d) -> p b d", b=8),
                            Dz[i][:, mc, s0:s0 + 8][:, :, None]
                            .broadcast_to([128, 8, D]), MULT)
                    J = Jn
                # A = W_out @ J  -> [64, 8*64]; zng2 block = row sums of A*A
                pA = psa.tile([64, JN], F32, tag="pA")
                for kc in range(NMC):
                    nc.tensor.matmul(pA[:], RWout[:, kc, :],
                                     J[:, kc, :, :].rearrange("p b d -> p (b d)"),
                                     start=(kc == 0), stop=(kc == NMC - 1))
                sq = pa.tile([64, JN], F32, tag="sq", name="sq")
                nc.scalar.square(sq[:], pA[:])
                nc.vector.tensor_reduce(
                    zng2[:, s0:s0 + 8],
                    sq[:].rearrange("p (b d) -> p b d", b=8),
                    mybir.AxisListType.X, mybir.AluOpType.add)
                if g % 16 == 15:
                    zng_flush(g // 16)

            for mg in range(4):
                nc.sync.dma_start(out_d.ap()[mg * 128:(mg + 1) * 128, :], O[mg][:])

    nc.compile()
    return nc


def _pack_consts(W_in, b_in, W_h, b_h, W_out, b_out):
    cb = np.zeros((128, CB_COLS), dtype=np.float32)
    cb[:, _O_WOUTR:_O_WOUTR + 256] = (
        W_out.T.reshape(4, 128, D).transpose(1, 0, 2).reshape(128, 256))
    cb[:, _O_BIN:_O_BIN + 4] = b_in.reshape(4, 128).T
    cb[:, _O_BH:_O_BH + 12] = b_h.reshape(3, 4, 128).transpose(2, 0, 1).reshape(128, 12)
    cb[:, _O_BOUTR:_O_BOUTR + 64] = np.tile(b_out, (128, 1))
    cb[:, _O_ID:_O_ID + 128] = np.eye(128, dtype=np.float32)

    # W_h[i]^T rearranged to [128, 4, 512] then hi/lo split on the fp32r grid
    WT = np.transpose(W_h, (0, 2, 1)).reshape(3, 4, 128, H).transpose(0, 2, 1, 3)
    WT = WT.reshape(3, 128, 2048)
    if JAC_F32R:
        WT_hi = _round_fp32r(WT)
        WT_lo = _round_fp32r(WT - WT_hi)
    else:
        WT_hi = WT
        WT_lo = np.zeros_like(WT)
    winr = W_in.reshape(4, 128, D).transpose(1, 0, 2).reshape(128, 256)
    woutr = W_out.T.reshape(4, 128, D).transpose(1, 0, 2).reshape(128, 256)
    if JAC_F32R:
        winr = _round_fp32r(winr)
        woutr = _round_fp32r(woutr)

    wint = W_in.T
    if JAC_F32R:
        wint_hi = _round_fp32r(wint)
        wint_lo = _round_fp32r(wint - wint_hi)
    else:
        wint_hi, wint_lo = wint, np.zeros_like(wint)
    cbr1a = np.zeros((128, CBR1A_COLS), dtype=np.float32)
    cbr1a[:, _O_RWT0:_O_RWT0 + 2048] = WT_hi[0]
    cbr1a[:, _O_WINR:_O_WINR + 256] = winr
    cbr1a[0:64, _O_WINTH:_O_WINTH + 512] = wint_hi
    cbr1a[0:64, _O_WINTL:_O_WINTL + 512] = wint_lo
    cbr1b = np.ascontiguousarray(WT_lo[0])

    cbr2a = np.zeros((128, CBR2A_COLS), dtype=np.float32)
    cbr2a[:, _O_RWT1:_O_RWT1 + 2048] = WT_hi[1]
    cbr2a[:, _O_WLO1:_O_WLO1 + 2048] = WT_lo[1]
    cbr2b = np.zeros((128, CBR2B_COLS), dtype=np.float32)
    cbr2b[:, _O_RWT2:_O_RWT2 + 2048] = WT_hi[2]
    cbr2b[:, _O_WLO2:_O_WLO2 + 2048] = WT_lo[2]
    cbr2b[:, _O_RWOUT:_O_RWOUT + 256] = woutr
    return cb, cbr1a, cbr1b, cbr2a, cbr2b


_CACHE = {}


def _get_nc():
    key = ("nc", JAC_F32R)
    if key not in _CACHE:
        _CACHE[key] = _build()
    return _CACHE[key]


def kernel(x, W_in, b_in, W_h, b_h, W_out, b_out, _trace=False):
    x = np.asarray(x, dtype=np.float32)
    cb, cbr1a, cbr1b, cbr2a, cbr2b = _pack_consts(
        np.asarray(W_in, np.float32), np.asarray(b_in, np.float32),
        np.asarray(W_h, np.float32), np.asarray(b_h, np.float32),
        np.asarray(W_out, np.float32), np.asarray(b_out, np.float32))
    in_maps = []
    for c in range(N_CORES):
        sh = x[c * BC:(c + 1) * BC]          # [512, 192]
        xb = np.ascontiguousarray(
            np.concatenate([sh[:, 0:D].T, sh[:, D:2 * D].T, sh[:, 2 * D:].T],
                           axis=1))           # [64, 1536]
        if JAC_F32R:
            xbh = _round_fp32r(xb)
            xbl = _round_fp32r(xb - xbh)
        else:
            xbh, xbl = xb, np.zeros_like(xb)
        in_maps.append({"cb": cb, "cbr1a": cbr1a, "cbr1b": cbr1b,
                        "cbr2a": cbr2a, "cbr2b": cbr2b,
                        "xbh": xbh, "xbl": xbl})

    nc = _get_nc()
    res = run_bass_kernel_spmd(nc, in_maps, list(range(N_CORES)), trace=_trace)
    out = np.concatenate([res.results[c]["out"] for c in range(N_CORES)], axis=0)
    if _trace:
        kernel._last_results = res
    return out

